# revision 52
# baseline (speedup 1.0000x reference)
"""Trainium2 Bass kernel for a transformer decoder block (self-attn + cross-attn + MLP).

Sharding: 8 cores = 4 batches x 2 sequence-halves; each core computes the full
block for its 512 query tokens (k/v for self-attention over the full sequence on
every core; cross k/v over the full context likewise).

All activations are feature-major ([features, tokens], "T" suffix) so every
matmul contraction dim lands on SBUF partitions with zero on-device transposes:
  - projections:   out^T[f,t] = sum_d W^T[d,f] . h^T[d,t]     (W^T stationary)
  - v token-major: v[t,f]     = sum_d h^T[d,t] . Wv^T[d,f]    (h^T stationary)
  - scores^T[k,q] = sum_d K^T[d,k] . q^T[d,q]                 (K^T stationary)
  - att^T[d,q]    = sum_k [V|1][k,d] . P^T[k,q]               (V stationary; the
      appended ones column makes PSUM row 64 the softmax denominator)

v2 restructure (from trace analysis of v1 @ 827us):
  - v1 spent 213us of ACT on 256 per-(head,kt) Exp calls serialized against the
    PE (scores -> exp -> av chains), holding PE duty ~50% so the HAM clock gate
    never re-warmed (298us at 1.2GHz). v2 pipelines exp one kt behind the score
    matmuls and lags the av accumulation one kt behind exp, so PE streams.
  - causal triangle applied as a 0/1 fp16 *multiply* on the exp output (DVE,
    2x 16-bit rate) instead of fp32 bias-add before exp; the other-half tail
    keys use ACT's free per-partition bias operand (0 or -30000 per core).
  - cross k/v projections interleaved into the self-attention window to fill
    PE slack; LN chains restructured (stats matmuls lag-interleaved into the
    producing projection's PE stream) to kill phase-boundary bubbles.
  - LN stats + apply both run off the fp16 activation copy (no fp32 twin);
    A/B broadcast tiles in fp16 for 2x DVE apply rate. fp32 x kept only for
    the residual (own half).
  - PSUM pools are phase-scoped to fit 8 banks: dense phases use stats(2)+mm(2);
    attention phases use mm(2)+scores(4)+av(2).
  - q/k/q2 PSUM evictions moved to ACT (idle during projections); k2/v2/v
    evictions stay on DVE (ACT is exp-bound in attention windows).
  - warmup matmuls at t=0 cover the initial DMA+LN latency and trip the HAM
    clock gate to 2.4GHz before real work arrives; final output DMA'd per
    feature tile as fc2 finishes instead of one tail DMA.

Matmul operands are fp16 (1 cyc/row on the PE); accumulation is fp32 in PSUM
and the residual stream is fp32 in SBUF. LayerNorm gammas are folded into the
following projection weights on the host, and the softmax 1/sqrt(HD) into the
q-projection weights. Softmax runs without max-subtraction (scores are O(3)
for this problem's input distribution). Per-core token rotation puts each
core's own 512 tokens at columns 0..511 so one uniform SPMD program serves
both sequence halves.
"""

import numpy as np
from contextlib import ExitStack

import concourse.bass as bass
import concourse.tile as tile
from concourse import bacc, mybir
from concourse.bass_utils import run_bass_kernel_spmd

F32 = mybir.dt.float32
F16 = mybir.dt.float16
AFT = mybir.ActivationFunctionType
ALU = mybir.AluOpType

B, L, D = 4, 1024, 1024
MCTX = 1024
NH, HD = 16, 64
HID = 4 * D
EPS = 1e-6
SCALE = HD ** -0.5
Q = 512
P = 128
NEG = -30000.0

_CACHE = {}


def _ln_chain(nc, pp, ps_s, ps_q, width, tag):
    """Stats PSUM ([1,width] sum, sumsq) -> fp16 A/B broadcast tiles."""
    sc, bc = pp["stats"], pp["bcast"]
    m2 = sc.tile([1, width], F32, tag="sc_a", name=f"m2_{tag}")
    nc.scalar.activation(m2, ps_s, AFT.Square)
    v1 = sc.tile([1, width], F32, tag="sc_b", name=f"v1_{tag}")
    nc.vector.tensor_scalar(v1, m2, 1.0 / D, None, ALU.mult)
    v2 = sc.tile([1, width], F32, tag="sc_c", name=f"v2_{tag}")
    nc.vector.tensor_tensor(v2, ps_q, v1, ALU.subtract)
    st = sc.tile([1, width], F32, tag="sc_a", name=f"st_{tag}")
    nc.scalar.activation(st, v2, AFT.Sqrt, bias=pp["eps"], scale=1.0 / D)
    a = sc.tile([1, width], F32, tag="sc_b", name=f"a_{tag}")
    nc.vector.reciprocal_approx_fast(a, st)
    b0 = sc.tile([1, width], F32, tag="sc_c", name=f"b0_{tag}")
    nc.vector.tensor_mul(b0, ps_s, a)
    bb = sc.tile([1, width], F32, tag="sc_a", name=f"bb_{tag}")
    nc.vector.tensor_scalar(bb, b0, -1.0 / D, None, ALU.mult)
    af = sc.tile([1, width], F16, tag="sc_f16a", name=f"af_{tag}")
    nc.vector.tensor_copy(af, a)
    bf = sc.tile([1, width], F16, tag="sc_f16b", name=f"bf_{tag}")
    nc.vector.tensor_copy(bf, bb)
    A = bc.tile([P, width], F16, tag="A", bufs=1, name=f"A_{tag}")
    nc.gpsimd.partition_broadcast(A, af)
    Bt = bc.tile([P, width], F16, tag="Bt", bufs=1, name=f"Bt_{tag}")
    nc.gpsimd.partition_broadcast(Bt, bf)
    return A, Bt


def _ln(nc, pp, src16, out16, width, tag, gp_share=False):
    """LayerNorm over features: src16 [128, 8, width] fp16, out16 fp16.
    Stats via ones-matmuls (squares on ACT); apply via fp16 A/B broadcast
    (2x DVE rate), optionally alternating DVE/GPSIMD to balance load."""
    ones, psum, tmp = pp["ones"], pp["psum_stats"], pp["tmp"]
    for ch in range(width // 512):
        cs = slice(ch * 512, ch * 512 + 512)
        ps_s = psum.tile([1, 512], F32, tag="ps_s", name=f"ps_s_{tag}{ch}")
        ps_q = psum.tile([1, 512], F32, tag="ps_q", name=f"ps_q_{tag}{ch}")
        for dt in range(8):
            nc.tensor.matmul(ps_s, ones, src16[:, dt, cs],
                             start=(dt == 0), stop=(dt == 7))
            sq = tmp.tile([P, 512], F16, tag="sq")
            nc.scalar.activation(sq, src16[:, dt, cs], AFT.Square)
            nc.tensor.matmul(ps_q, ones, sq,
                             start=(dt == 0), stop=(dt == 7))
        A, Bt = _ln_chain(nc, pp, ps_s, ps_q, 512, f"{tag}{ch}")
        for dt in range(8):
            eng = nc.gpsimd if (gp_share and dt % 2) else nc.vector
            t1 = tmp.tile([P, 512], F16, tag="lnap")
            eng.tensor_mul(t1, src16[:, dt, cs], A)
            eng.tensor_add(out16[:, dt, cs], t1, Bt)


def _proj(nc, pp, w_dram, h_src, n_f_tiles, t_width, n_d_tiles=8):
    """Yields (ft, th, psum): out^T[f-tile] = sum_d W^T-tile . h_src tile."""
    wpool, psum = pp["wpool"], pp["psum_mm"]
    w_ap = w_dram.ap().rearrange("(dt dp) f -> dp dt f", dp=P)
    for c in range((n_f_tiles + 3) // 4):
        fw = min(512, (n_f_tiles - c * 4) * P)
        wc = wpool.tile([P, n_d_tiles, 512], F16, tag="w")
        nc.sync.dma_start(out=wc[:, :, :fw],
                          in_=w_ap[:, :, c * 512:c * 512 + fw])
        for fs in range(fw // P):
            ft = c * 4 + fs
            for th in range(t_width // 512):
                ps = psum.tile([P, 512], F32, tag="ps_mm")
                for dt in range(n_d_tiles):
                    nc.tensor.matmul(ps, wc[:, dt, fs * P:fs * P + P],
                                     h_src[:, dt, th * 512:th * 512 + 512],
                                     start=(dt == 0), stop=(dt == n_d_tiles - 1))
                yield ft, th, ps


def _vproj(nc, pp, w_dram, h_src, vt):
    """v[t, f] token-major with ones col at index 64: vt [128, 8, 16, 65].
    Yields after each (c, tt) slice (16 total) for interleaved emission."""
    wpool, psum = pp["wpool"], pp["psum_mm"]
    w_ap = w_dram.ap().rearrange("(dt dp) f -> dp dt f", dp=P)
    for c in range(2):
        wc = wpool.tile([P, 8, 512], F16, tag="w")
        nc.sync.dma_start(out=wc, in_=w_ap[:, :, c * 512:c * 512 + 512])
        for tt in range(8):
            ps = psum.tile([P, 512], F32, tag="ps_mm")
            for dt in range(8):
                nc.tensor.matmul(ps, h_src[:, dt, tt * P:tt * P + P],
                                 wc[:, dt, :], start=(dt == 0), stop=(dt == 7))
            nc.vector.tensor_copy(vt[:, tt, c * 8:c * 8 + 8, 0:HD],
                                  ps.rearrange("p (h d) -> p h d", h=8))
            yield


# fp16 Schraudolph exp on DVE: bits = round(1024*log2(e)*x + sigma),
# bitcast int16 -> fp16. Max rel err ~3%; softmax normalization cancels
# most of it (validated end-to-end: adds <1e-4 to final rel err).
# Input must be clamped to [-10, 9] first so the bits stay inside the
# finite-fp16 range [1, 31743] with no reliance on int16 saturation
# (exp(-10) ~ 4.5e-5 is 0 at softmax scale; scores never reach +9).
SEXP_A = 1477.3197
SEXP_B = 15316.5
SEXP_LO = -10.0
SEXP_HI = 9.0
AV_LAG = 3


def _attn_pair(nc, pp, hp, kT, vt, qT, out_sa, mk, tb, psum_sc, psum_av,
               dve_kts=()):
    """One head pair (2hp, 2hp+1) of attention, exp pipelined on ACT.
    Scores for the two heads are emitted adjacently (lhsT base partitions
    0/64) so the K=64 matmuls row-tile concurrently. av matmuls lag AV_LAG
    kts behind the scores so the PE never waits on ACT/DVE.
    mk: [128, 4, Q] fp16 0/1 causal mask for kt<4 (own half), or None.
    tb: [P,1] tail bias AP for kt>=4 (0 or -30000), or None.
    dve_kts: kts whose exp runs on DVE (Schraudolph); only for unmasked
    unbiased tiles (cross-attention)."""
    h0, h1 = 2 * hp, 2 * hp + 1
    tmp, sc_pool, bc = pp["tmp"], pp["stats"], pp["bcast"]
    avs = {}
    for h in (h0, h1):
        avs[h] = psum_av.tile([P, Q], F32, tag="av", name=f"av_{h}")
    pex = {}

    def emit_av(h, kt):
        nc.tensor.matmul(avs[h][0:HD + 1, :], vt[:, kt, h, :], pex.pop((h, kt)),
                         start=(kt == 0), stop=(kt == 7))

    for kt in range(8):
        scs = {}
        for h, fo in ((h0, 0), (h1, HD)):
            sc = psum_sc.tile([P, Q], F32, tag="sc", name=f"sc_{h}_{kt}")
            nc.tensor.matmul(sc, kT[fo:fo + HD, hp, kt * P:kt * P + P],
                             qT[fo:fo + HD, hp, :], start=True, stop=True)
            scs[h] = sc
        for h in (h0, h1):
            if kt in dve_kts:
                pxc = tmp.tile([P, Q], F32, tag="pexp_c", bufs=1,
                               name=f"pexpc_{h}_{kt}")
                nc.vector.tensor_scalar(pxc, scs[h], SEXP_HI, SEXP_LO,
                                        ALU.min, ALU.max)
                pxi = tmp.tile([P, Q], mybir.dt.int16, tag="pexp_i", bufs=4,
                               name=f"pexpi_{h}_{kt}")
                nc.vector.tensor_scalar(pxi, pxc, SEXP_A, SEXP_B,
                                        ALU.mult, ALU.add)
                pex[(h, kt)] = pxi.bitcast(F16)
                continue
            px_ = tmp.tile([P, Q], F16, tag="pexp", bufs=2 * AV_LAG + 2,
                           name=f"pexp_{h}_{kt}")
            if mk is not None and kt < 4:
                nc.scalar.activation(px_, scs[h], AFT.Exp)
                nc.vector.tensor_mul(px_, px_, mk[:, kt, :])
            elif tb is not None and kt >= 4:
                nc.scalar.activation(px_, scs[h], AFT.Exp, bias=tb)
            else:
                nc.scalar.activation(px_, scs[h], AFT.Exp)
            pex[(h, kt)] = px_
        if kt >= AV_LAG:
            for h in (h0, h1):
                emit_av(h, kt - AV_LAG)
    for kt in range(8 - AV_LAG, 8):
        for h in (h0, h1):
            emit_av(h, kt)

    for i, (h, fo) in enumerate(((h0, 0), (h1, HD))):
        # evict av to SBUF right away so the PSUM bank frees for the next
        # pair without waiting on the normalize chain; the denominator row
        # goes to its own partition-0 tile (custom-DVE recip and gpsimd
        # broadcast read from partition 0)
        avsb = tmp.tile([HD, Q], F32, tag="avsb", bufs=2, name=f"avsb_{h}")
        nc.vector.tensor_copy(avsb, avs[h][0:HD, :])
        dn = sc_pool.tile([1, Q], F32, tag="sc_c", name=f"dn_{h}")
        nc.vector.tensor_copy(dn, avs[h][HD:HD + 1, :])
        r = sc_pool.tile([1, Q], F32, tag="recip", name=f"r_{h}")
        nc.vector.reciprocal_approx_fast(r, dn)
        rb = bc.tile([HD, Q], F32, tag="rb", name=f"rb_{h}")
        nc.gpsimd.partition_broadcast(rb, r)
        nc.vector.tensor_mul(out_sa[fo:fo + HD, hp, :], avsb, rb)


def build_program():
    nc = bacc.Bacc("TRN2", target_bir_lowering=False, debug=False,
                   enable_asserts=False)

    din = lambda n, shape, dt_=F16: nc.declare_dram_parameter(
        n, shape, dt_, isOutput=False)
    x16 = din("x16", [D, L])             # fp16, rotated (stats + LN apply)
    xres = din("xres", [D, Q], F32)      # fp32 own half (residual stream)
    ctx16 = din("ctx16", [D, MCTX])
    maskT = din("maskT", [Q, Q])         # own-half causal 0/1 fp16, [keys, q]
    tbias = din("tbias", [P, 1], F32)    # 0 (s=1) or -30000 (s=0) tail bias
    WqT, WkT, WvT = din("WqT", [D, D]), din("WkT", [D, D]), din("WvT", [D, D])
    WsoT, Wq2T = din("WsoT", [D, D]), din("Wq2T", [D, D])
    Wk2T, Wv2T = din("Wk2T", [D, D]), din("Wv2T", [D, D])
    WcoT = din("WcoT", [D, D])
    W1T, W2T = din("W1T", [D, HID]), din("W2T", [HID, D])
    outT = nc.declare_dram_parameter("outT", [D, Q], F32, isOutput=True)

    es = {}
    with tile.TileContext(nc) as tc, ExitStack() as top:
        def popen(name, side=None, bufs=1, space=None):
            s = ExitStack()
            es[name] = s
            kwargs = dict(name=name, bufs=bufs)
            if side is not None:
                kwargs["side"] = side
            if space is not None:
                kwargs["space"] = space
            return s.enter_context(tc.tile_pool(**kwargs))

        def pclose(name):
            es.pop(name).close()

        const = top.enter_context(tc.tile_pool(name="const", bufs=1))
        wpool = top.enter_context(tc.tile_pool(name="wpool", bufs=2))
        tmp = top.enter_context(tc.tile_pool(name="tmp", bufs=2))
        stats = top.enter_context(tc.tile_pool(name="stats", bufs=1))
        bcast = top.enter_context(tc.tile_pool(name="bcast", bufs=2))
        psum_mm = top.enter_context(
            tc.tile_pool(name="psum_mm", bufs=2, space="PSUM"))

        ones = const.tile([P, 1], F16)
        nc.vector.memset(ones.bitcast(mybir.dt.uint16), 15360)
        eps_t = const.tile([1, 1], F32)
        nc.vector.memset(eps_t, EPS)
        tb_t = const.tile([P, 1], F32)
        nc.sync.dma_start(out=tb_t, in_=tbias[:, :])
        wjunk = const.tile([P, 512], F16)
        nc.vector.memset(wjunk.bitcast(mybir.dt.uint16), 0)

        pp = {"ones": ones, "eps": eps_t, "wpool": wpool, "tmp": tmp,
              "stats": stats, "bcast": bcast, "psum_mm": psum_mm}

        x16_r = x16.ap().rearrange("(dt dp) t -> dp dt t", dp=P)
        c16_r = ctx16.ap().rearrange("(dt dp) t -> dp dt t", dp=P)
        mask_r = maskT.ap().rearrange("(kt kp) q -> kp kt q", kp=P)

        # ---- warmup: keep PE busy + HAM warm through initial DMA/LN -------
        for wi in range(14):
            psw = psum_mm.tile([P, 512], F32, tag="ps_mm", name=f"warm{wi}")
            nc.tensor.matmul(psw[0:1, :], ones, wjunk, start=True, stop=True)

        # ---- phase A: norm1 + ctx norm + qkv ------------------------------
        pp["psum_stats"] = popen("psum_stats", space="PSUM")

        pcatt1 = popen("pcatt1", "left")
        k2T = pcatt1.tile([P, 8, MCTX], F16, tag="k2T")
        v2t = pcatt1.tile([P, 8, NH, HD + 1], F16, tag="v2t")
        nc.gpsimd.memset(v2t.bitcast(mybir.dt.uint16), 15360)
        phc = popen("phc", "left")
        hc = phc.tile([P, 8, MCTX], F16, tag="hc")
        pattn2 = popen("pattn2", "left")
        mk = pattn2.tile([P, 4, Q], F16, tag="mk")
        sa = pattn2.tile([P, 8, Q], F16, tag="sa")
        pattn1 = popen("pattn1", "left")
        qT = pattn1.tile([P, 8, Q], F16, tag="qT")
        kT = pattn1.tile([P, 8, L], F16, tag="kT")
        vt = pattn1.tile([P, 8, NH, HD + 1], F16, tag="vt")
        nc.gpsimd.memset(vt.bitcast(mybir.dt.uint16), 15360)
        pctx = popen("pctx", "left")
        cs16 = pctx.tile([P, 8, MCTX], F16, tag="cs16")
        px = popen("px", "left")
        xs = px.tile([P, 8, L], F16, tag="xs")
        nc.sync.dma_start(out=xs[:, :, 0:512], in_=x16_r[:, :, 0:512])
        nc.sync.dma_start(out=xs[:, :, 512:1024], in_=x16_r[:, :, 512:1024])
        nc.sync.dma_start(out=cs16, in_=c16_r)
        nc.sync.dma_start(out=mk, in_=mask_r)

        pht = popen("pht", "right")
        ht = pht.tile([P, 8, L], F16, tag="ht")
        _ln(nc, pp, xs, ht, L, "x")
        for wi in range(10):  # always-ready fillers: bridge the LN-chain
            psw = psum_mm.tile([P, 512], F32, tag="ps_mm", name=f"warmA{wi}")
            nc.tensor.matmul(psw[0:1, :], ones, wjunk, start=True, stop=True)
        pclose("px")
        # k first (self-attn pair 0 blocks on kT/qT ft0); evictions on ACT
        # (DVE carries the LN applies and vt evictions in this phase).
        # ctx LN emitted after the q evictions so B's first scores don't
        # queue behind its ACT squares/chain.
        for ft, th, ps in _proj(nc, pp, WkT, ht, 8, L):
            nc.scalar.copy(kT[:, ft, th * 512:th * 512 + 512], ps)
        for ft, th, ps in _proj(nc, pp, WqT, ht, 8, Q):
            nc.scalar.copy(qT[:, ft, :], ps)
        _ln(nc, pp, cs16, hc, MCTX, "c")
        pclose("pctx")
        for _ in _vproj(nc, pp, WvT, ht, vt):
            pass
        pclose("pht")
        pclose("psum_stats")

        # ---- phase B: self-attention, cross k/v interleaved ---------------
        k2_gen = _proj(nc, pp, Wk2T, hc, 8, MCTX)
        v2_gen = _vproj(nc, pp, Wv2T, hc, v2t)

        def k2_step(n):
            for _ in range(n):
                ft, th, ps = next(k2_gen, (None, None, None))
                if ps is None:
                    return
                nc.vector.tensor_copy(k2T[:, ft, th * 512:th * 512 + 512], ps)

        def v2_step(n):
            for _ in range(n):
                if next(v2_gen, "end") == "end":
                    return

        psum_sc = popen("psum_sc", space="PSUM", bufs=4)
        psum_av = popen("psum_av", space="PSUM", bufs=2)

        for hp in range(NH // 2):
            _attn_pair(nc, pp, hp, kT, vt, qT, sa, mk, tb_t, psum_sc, psum_av)
            # fill PE slack with cross k/v projection slices. NOTE: k2 and
            # v2 share the wpool "w" ring (bufs=2); consuming 2+2 per pair
            # keeps their chunk DMAs from reusing a buffer whose reads are
            # still being emitted (1+2 consumption raced and NaN'd).
            k2_step(2)
            v2_step(2)
        pclose("psum_av")
        pclose("psum_sc")
        pclose("pattn1")

        # ---- phase C: self out-proj + residual + LN(xa) + q2 --------------
        k2_step(16)
        v2_step(16)
        pp["psum_stats"] = psum_stats = popen("psum_stats", space="PSUM")
        ps_s = psum_stats.tile([1, Q], F32, tag="ps_s", name="ps_s_a")
        ps_q = psum_stats.tile([1, Q], F32, tag="ps_q", name="ps_q_a")

        pxa = popen("pxa", "right")
        resid = pxa.tile([P, 8, Q], F32, tag="resid")
        nc.sync.dma_start(
            out=resid, in_=xres.ap().rearrange("(dt dp) t -> dp dt t", dp=P))
        xa = pxa.tile([P, 8, Q], F32, tag="xa")
        xa16 = pxa.tile([P, 8, Q], F16, tag="xa16")
        stat_lag = []
        for ft, th, ps in _proj(nc, pp, WsoT, sa, 8, Q):
            nc.vector.tensor_add(xa[:, ft, :], ps, resid[:, ft, :])
            nc.scalar.copy(xa16[:, ft, :], xa[:, ft, :])
            sq = tmp.tile([P, Q], F16, tag="sq")
            nc.vector.tensor_mul(sq, xa16[:, ft, :], xa16[:, ft, :])
            stat_lag.append((ft, sq))
            if len(stat_lag) >= 2:  # lag stats MMs so PE doesn't stall
                lft, lsq = stat_lag.pop(0)
                nc.tensor.matmul(ps_s, ones, xa16[:, lft, :],
                                 start=(lft == 0), stop=(lft == 7))
                nc.tensor.matmul(ps_q, ones, lsq,
                                 start=(lft == 0), stop=(lft == 7))
        for lft, lsq in stat_lag:
            nc.tensor.matmul(ps_s, ones, xa16[:, lft, :],
                             start=(lft == 0), stop=(lft == 7))
            nc.tensor.matmul(ps_q, ones, lsq, start=(lft == 0), stop=(lft == 7))
        pclose("pattn2")
        pclose("phc")
        for wi in range(10):
            psw = psum_mm.tile([P, 512], F32, tag="ps_mm", name=f"warmC{wi}")
            nc.tensor.matmul(psw[0:1, :], ones, wjunk, start=True, stop=True)
        A_, B_ = _ln_chain(nc, pp, ps_s, ps_q, Q, "xa")
        pq2 = popen("pq2", "left")
        hq = pq2.tile([P, 8, Q], F16, tag="hq")
        for dt in range(8):
            t1 = tmp.tile([P, Q], F16, tag="lnap")
            nc.vector.tensor_mul(t1, xa16[:, dt, :], A_)
            nc.vector.tensor_add(hq[:, dt, :], t1, B_)
        q2T = pq2.tile([P, 8, Q], F16, tag="q2T")
        for ft, th, ps in _proj(nc, pp, Wq2T, hq, 8, Q):
            nc.scalar.copy(q2T[:, ft, :], ps)
        pclose("psum_stats")

        # ---- phase D: cross-attention, out-proj ft0/1 interleaved ---------
        psum_sc = popen("psum_sc", space="PSUM", bufs=4)
        psum_av = popen("psum_av", space="PSUM", bufs=2)
        ca = pq2.tile([P, 8, Q], F16, tag="ca")
        co_ap = WcoT.ap().rearrange("(dt dp) f -> dp dt f", dp=P)
        wc_co = wpool.tile([P, 8, 512], F16, tag="w", name="wc_co0")
        nc.sync.dma_start(out=wc_co, in_=co_ap[:, :, 0:512])
        co01 = [psum_mm.tile([P, Q], F32, tag="ps_mm", name=f"co{f}")
                for f in range(2)]

        def co_mm(dt):
            for f in range(2):
                nc.tensor.matmul(co01[f], wc_co[:, dt, f * P:f * P + P],
                                 ca[:, dt, :], start=(dt == 0), stop=(dt == 7))

        for hp in range(NH // 2):
            _attn_pair(nc, pp, hp, k2T, v2t, q2T, ca, None, None,
                       psum_sc, psum_av, dve_kts=(5,))
            if hp >= 1:  # lag: ca[:, hp-1, :] is settled by now
                co_mm(hp - 1)
        co_mm(7)
        pclose("psum_av")
        pclose("psum_sc")

        # ---- phase D2: cross out-proj rest + residual + LN(xb) ------------
        pp["psum_stats"] = psum_stats = popen("psum_stats", space="PSUM")
        ps_s = psum_stats.tile([1, Q], F32, tag="ps_s", name="ps_s_b")
        ps_q = psum_stats.tile([1, Q], F32, tag="ps_q", name="ps_q_b")
        pxb = popen("pxb", "right")
        xb = pxb.tile([P, 8, Q], F32, tag="xb")
        xb16 = pxb.tile([P, 8, Q], F16, tag="xb16")

        psum_co = popen("psum_co", space="PSUM", bufs=2)
        for wi in range(4):
            psw = psum_co.tile([P, 512], F32, tag="ps_co", name=f"warmD{wi}")
            nc.tensor.matmul(psw[0:1, :], ones, wjunk, start=True, stop=True)

        def co_rest():
            yield 0, co01[0]
            yield 1, co01[1]
            for ft in (2, 3):
                ps = psum_co.tile([P, Q], F32, tag="ps_co", name=f"co{ft}")
                for dt in range(8):
                    nc.tensor.matmul(ps, wc_co[:, dt, ft * P:ft * P + P],
                                     ca[:, dt, :],
                                     start=(dt == 0), stop=(dt == 7))
                yield ft, ps
            wc_co1 = wpool.tile([P, 8, 512], F16, tag="w", name="wc_co1")
            nc.sync.dma_start(out=wc_co1, in_=co_ap[:, :, 512:1024])
            for ft in (4, 5, 6, 7):
                ps = psum_co.tile([P, Q], F32, tag="ps_co", name=f"co{ft}")
                for dt in range(8):
                    nc.tensor.matmul(
                        ps, wc_co1[:, dt, (ft - 4) * P:(ft - 4) * P + P],
                        ca[:, dt, :], start=(dt == 0), stop=(dt == 7))
                yield ft, ps

        stat_lag = []
        for ft, ps in co_rest():
            nc.vector.tensor_add(xb[:, ft, :], ps, xa[:, ft, :])
            nc.scalar.copy(xb16[:, ft, :], xb[:, ft, :])
            sq = tmp.tile([P, Q], F16, tag="sq")
            nc.vector.tensor_mul(sq, xb16[:, ft, :], xb16[:, ft, :])
            stat_lag.append((ft, sq))
            if len(stat_lag) >= 2:
                lft, lsq = stat_lag.pop(0)
                nc.tensor.matmul(ps_s, ones, xb16[:, lft, :],
                                 start=(lft == 0), stop=(lft == 7))
                nc.tensor.matmul(ps_q, ones, lsq,
                                 start=(lft == 0), stop=(lft == 7))
        for lft, lsq in stat_lag:
            nc.tensor.matmul(ps_s, ones, xb16[:, lft, :],
                             start=(lft == 0), stop=(lft == 7))
            nc.tensor.matmul(ps_q, ones, lsq, start=(lft == 0), stop=(lft == 7))
        pclose("psum_co")
        pclose("pq2")
        pclose("pcatt1")

        for wi in range(10):
            psw = psum_mm.tile([P, 512], F32, tag="ps_mm", name=f"warmE{wi}")
            nc.tensor.matmul(psw[0:1, :], ones, wjunk, start=True, stop=True)
        A_, B_ = _ln_chain(nc, pp, ps_s, ps_q, Q, "xb")
        pmlp = popen("pmlp", "left")
        h2 = pmlp.tile([P, 8, Q], F16, tag="h2")
        for dt in range(8):
            t1 = tmp.tile([P, Q], F16, tag="lnap")
            nc.vector.tensor_mul(t1, xb16[:, dt, :], A_)
            nc.vector.tensor_add(h2[:, dt, :], t1, B_)
        pclose("psum_stats")

        # ---- phase E: MLP + streamed output -------------------------------
        gt = pmlp.tile([P, 32, Q], F16, tag="gt")
        for ft, th, ps in _proj(nc, pp, W1T, h2, 32, Q):
            nc.scalar.activation(gt[:, ft, :], ps, AFT.Gelu)

        out_r = outT.ap().rearrange("(dt dp) q -> dp dt q", dp=P)
        w2_ap = W2T.ap().rearrange("(dt dp) f -> dp dt f", dp=P)
        pw2 = popen("pw2", "left")
        for fh in range(4):
            pss = [psum_mm.tile([P, Q], F32, tag="ps_mm", name=f"fc2_{fh}_{e}")
                   for e in range(2)]
            for g in range(4):
                wc = pw2.tile([P, 8, 256], F16, tag="w2", bufs=3,
                              name=f"w2_{fh}_{g}")
                nc.sync.dma_start(
                    out=wc,
                    in_=w2_ap[:, g * 8:g * 8 + 8, fh * 256:fh * 256 + 256])
                for e in range(2):
                    for dt in range(8):
                        nc.tensor.matmul(pss[e], wc[:, dt, e * P:e * P + P],
                                         gt[:, g * 8 + dt, :],
                                         start=(g == 0 and dt == 0),
                                         stop=(g == 3 and dt == 7))
            for e in range(2):
                et = fh * 2 + e
                ot = tmp.tile([P, Q], F32, tag="ot", bufs=2, name=f"ot_{et}")
                nc.vector.tensor_add(ot, pss[e], xb[:, et, :])
                nc.sync.dma_start(out=out_r[:, et, :], in_=ot)
        pclose("pxb")
        pclose("pxa")
        pclose("pw2")
        pclose("pmlp")

    nc.compile()
    return nc


# ----------------------------------------------------------------------------
# host side
# ----------------------------------------------------------------------------

def _prep_inputs(x, context, sa_mask, W_qkv, W_self_out, W_q, W_kv, W_cross_out,
                 W_fc1, W_fc2, g_norm1, g_query_norm, g_context_norm, g_norm2):
    f32, f16 = np.float32, np.float16
    g1 = np.asarray(g_norm1, f32)[:, None]
    gq = np.asarray(g_query_norm, f32)[:, None]
    gc = np.asarray(g_context_norm, f32)[:, None]
    g2 = np.asarray(g_norm2, f32)[:, None]
    W_qkv = np.asarray(W_qkv, f32)
    W_kv = np.asarray(W_kv, f32)
    cw = lambda a: np.ascontiguousarray(a.astype(f16))
    weights = {
        "WqT": cw(W_qkv[0:D].T * g1 * f32(SCALE)),
        "WkT": cw(W_qkv[D:2 * D].T * g1),
        "WvT": cw(W_qkv[2 * D:3 * D].T * g1),
        "WsoT": cw(np.asarray(W_self_out, f32).T),
        "Wq2T": cw(np.asarray(W_q, f32).T * gq * f32(SCALE)),
        "Wk2T": cw(W_kv[0:D].T * gc),
        "Wv2T": cw(W_kv[D:2 * D].T * gc),
        "WcoT": cw(np.asarray(W_cross_out, f32).T),
        "W1T": cw(np.asarray(W_fc1, f32).T * g2),
        "W2T": cw(np.asarray(W_fc2, f32).T),
    }
    in_maps = []
    for c in range(8):
        b, s = c // 2, c % 2
        own = np.arange(s * Q, s * Q + Q)
        idx = np.concatenate([own, np.arange((1 - s) * Q, (1 - s) * Q + Q)])
        xb = np.asarray(x[b], f32)
        mask01 = np.where(np.asarray(sa_mask[b])[np.ix_(own, own)] == 0,
                          f16(0.0), f16(1.0))
        m = dict(weights)
        xr = np.ascontiguousarray(xb[idx].T)
        m["x16"] = xr.astype(f16)
        m["xres"] = np.ascontiguousarray(xr[:, 0:Q])
        m["maskT"] = np.ascontiguousarray(mask01.T.astype(f16))
        m["tbias"] = np.full((P, 1), NEG if s == 0 else 0.0, f32)
        m["ctx16"] = np.ascontiguousarray(
            np.asarray(context[b], f32).T.astype(f16))
        in_maps.append(m)
    return in_maps


def _check_mask(sa_mask):
    """Fast program assumes causal block structure across the two halves:
    second-half keys all-masked for first-half queries, all-open for
    second-half queries."""
    mask = np.asarray(sa_mask)
    lo, hi = np.arange(0, Q), np.arange(Q, L)
    for b in range(B):
        if not np.all(mask[b][np.ix_(lo, hi)] == 0):
            return False
        if not np.all(mask[b][np.ix_(hi, lo)] != 0):
            return False
    return True


def _gather(results, x_dtype):
    out = np.empty((B, L, D), np.float32)
    for c in range(8):
        b, s = c // 2, c % 2
        out[b, s * Q:(s + 1) * Q, :] = results[c]["outT"].T
    return out.astype(x_dtype, copy=False)


def _run(trace=False, **inputs):
    assert _check_mask(inputs["sa_mask"]), \
        "sa_mask does not have the expected causal block structure"
    if "nc" not in _CACHE:
        _CACHE["nc"] = build_program()
    nc = _CACHE["nc"]
    in_maps = _prep_inputs(**inputs)
    res = run_bass_kernel_spmd(nc, in_maps, list(range(8)), trace=trace)
    out = _gather(res.results, np.asarray(inputs["x"]).dtype)
    return out, res


def kernel(**inputs) -> np.ndarray:
    out, _ = _run(trace=False, **inputs)
    return out


def kernel_traced(**inputs):
    """Returns (output, exec_time_ns). Used by test.py."""
    import sys, types
    try:
        import antenv
        import trn_agent_boot.trn_boot as tb
        import concourse.bass_utils as bu
        if "antenv.axon_hooks" not in sys.modules:
            hook = tb._ntff_profile_via_ctypes('/opt/axon/libaxon_pjrt.so')
            mod = types.ModuleType("antenv.axon_hooks")
            mod.get_axon_ntff_profile_hook = lambda: hook
            mod.set_axon_ntff_profile_hook = lambda h: None
            sys.modules['antenv.axon_hooks'] = mod
            antenv.axon_hooks = mod
        bu.upload_artifacts = lambda tmpdir: "local://skipped"
    except Exception as e:
        print(f"ntff hook install failed: {e}")
    out, res = _run(trace=True, **inputs)
    return out, res.exec_time_ns


# revision 53
# speedup vs baseline: 1.0128x; 1.0128x over previous
"""Trainium2 Bass kernel for a transformer decoder block (self-attn + cross-attn + MLP).

Sharding: 8 cores = 4 batches x 2 sequence-halves; each core computes the full
block for its 512 query tokens (k/v for self-attention over the full sequence on
every core; cross k/v over the full context likewise).

All activations are feature-major ([features, tokens], "T" suffix) so every
matmul contraction dim lands on SBUF partitions with zero on-device transposes:
  - projections:   out^T[f,t] = sum_d W^T[d,f] . h^T[d,t]     (W^T stationary)
  - v token-major: v[t,f]     = sum_d h^T[d,t] . Wv^T[d,f]    (h^T stationary)
  - scores^T[k,q] = sum_d K^T[d,k] . q^T[d,q]                 (K^T stationary)
  - att^T[d,q]    = sum_k [V|1][k,d] . P^T[k,q]               (V stationary; the
      appended ones column makes PSUM row 64 the softmax denominator)

v2 restructure (from trace analysis of v1 @ 827us):
  - v1 spent 213us of ACT on 256 per-(head,kt) Exp calls serialized against the
    PE (scores -> exp -> av chains), holding PE duty ~50% so the HAM clock gate
    never re-warmed (298us at 1.2GHz). v2 pipelines exp one kt behind the score
    matmuls and lags the av accumulation one kt behind exp, so PE streams.
  - causal triangle applied as a 0/1 fp16 *multiply* on the exp output (DVE,
    2x 16-bit rate) instead of fp32 bias-add before exp; the other-half tail
    keys use ACT's free per-partition bias operand (0 or -30000 per core).
  - cross k/v projections interleaved into the self-attention window to fill
    PE slack; LN chains restructured (stats matmuls lag-interleaved into the
    producing projection's PE stream) to kill phase-boundary bubbles.
  - LN stats + apply both run off the fp16 activation copy (no fp32 twin);
    A/B broadcast tiles in fp16 for 2x DVE apply rate. fp32 x kept only for
    the residual (own half).
  - PSUM pools are phase-scoped to fit 8 banks: dense phases use stats(2)+mm(2);
    attention phases use mm(2)+scores(4)+av(2).
  - q/k/q2 PSUM evictions moved to ACT (idle during projections); k2/v2/v
    evictions stay on DVE (ACT is exp-bound in attention windows).
  - warmup matmuls at t=0 cover the initial DMA+LN latency and trip the HAM
    clock gate to 2.4GHz before real work arrives; final output DMA'd per
    feature tile as fc2 finishes instead of one tail DMA.

Matmul operands are fp16 (1 cyc/row on the PE); accumulation is fp32 in PSUM
and the residual stream is fp32 in SBUF. LayerNorm gammas are folded into the
following projection weights on the host, and the softmax 1/sqrt(HD) into the
q-projection weights. Softmax runs without max-subtraction (scores are O(3)
for this problem's input distribution). Per-core token rotation puts each
core's own 512 tokens at columns 0..511 so one uniform SPMD program serves
both sequence halves.
"""

import numpy as np
from contextlib import ExitStack

import concourse.bass as bass
import concourse.tile as tile
from concourse import bacc, mybir
from concourse.bass_utils import run_bass_kernel_spmd

F32 = mybir.dt.float32
F16 = mybir.dt.float16
AFT = mybir.ActivationFunctionType
ALU = mybir.AluOpType

B, L, D = 4, 1024, 1024
MCTX = 1024
NH, HD = 16, 64
HID = 4 * D
EPS = 1e-6
SCALE = HD ** -0.5
Q = 512
P = 128
NEG = -30000.0

_CACHE = {}


def _ln_chain(nc, pp, ps_s, ps_q, width, tag):
    """Stats PSUM ([1,width] sum, sumsq) -> fp16 A/B broadcast tiles."""
    sc, bc = pp["stats"], pp["bcast"]
    m2 = sc.tile([1, width], F32, tag="sc_a", name=f"m2_{tag}")
    nc.scalar.activation(m2, ps_s, AFT.Square)
    v1 = sc.tile([1, width], F32, tag="sc_b", name=f"v1_{tag}")
    nc.vector.tensor_scalar(v1, m2, 1.0 / D, None, ALU.mult)
    v2 = sc.tile([1, width], F32, tag="sc_c", name=f"v2_{tag}")
    nc.vector.tensor_tensor(v2, ps_q, v1, ALU.subtract)
    st = sc.tile([1, width], F32, tag="sc_a", name=f"st_{tag}")
    nc.scalar.activation(st, v2, AFT.Sqrt, bias=pp["eps"], scale=1.0 / D)
    a = sc.tile([1, width], F32, tag="sc_b", name=f"a_{tag}")
    nc.vector.reciprocal_approx_fast(a, st)
    b0 = sc.tile([1, width], F32, tag="sc_c", name=f"b0_{tag}")
    nc.vector.tensor_mul(b0, ps_s, a)
    bb = sc.tile([1, width], F32, tag="sc_a", name=f"bb_{tag}")
    nc.vector.tensor_scalar(bb, b0, -1.0 / D, None, ALU.mult)
    af = sc.tile([1, width], F16, tag="sc_f16a", name=f"af_{tag}")
    nc.vector.tensor_copy(af, a)
    bf = sc.tile([1, width], F16, tag="sc_f16b", name=f"bf_{tag}")
    nc.vector.tensor_copy(bf, bb)
    A = bc.tile([P, width], F16, tag="A", bufs=1, name=f"A_{tag}")
    nc.gpsimd.partition_broadcast(A, af)
    Bt = bc.tile([P, width], F16, tag="Bt", bufs=1, name=f"Bt_{tag}")
    nc.gpsimd.partition_broadcast(Bt, bf)
    return A, Bt


def _ln(nc, pp, src16, out16, width, tag, gp_share=False):
    """LayerNorm over features: src16 [128, 8, width] fp16, out16 fp16.
    Stats via ones-matmuls (squares on ACT); apply via fp16 A/B broadcast
    (2x DVE rate), optionally alternating DVE/GPSIMD to balance load."""
    ones, psum, tmp = pp["ones"], pp["psum_stats"], pp["tmp"]
    for ch in range(width // 512):
        cs = slice(ch * 512, ch * 512 + 512)
        ps_s = psum.tile([1, 512], F32, tag="ps_s", name=f"ps_s_{tag}{ch}")
        ps_q = psum.tile([1, 512], F32, tag="ps_q", name=f"ps_q_{tag}{ch}")
        for dt in range(8):
            nc.tensor.matmul(ps_s, ones, src16[:, dt, cs],
                             start=(dt == 0), stop=(dt == 7))
            sq = tmp.tile([P, 512], F16, tag="sq")
            nc.scalar.activation(sq, src16[:, dt, cs], AFT.Square)
            nc.tensor.matmul(ps_q, ones, sq,
                             start=(dt == 0), stop=(dt == 7))
        A, Bt = _ln_chain(nc, pp, ps_s, ps_q, 512, f"{tag}{ch}")
        for dt in range(8):
            eng = nc.gpsimd if (gp_share and dt % 2) else nc.vector
            t1 = tmp.tile([P, 512], F16, tag="lnap")
            eng.tensor_mul(t1, src16[:, dt, cs], A)
            eng.tensor_add(out16[:, dt, cs], t1, Bt)


def _proj(nc, pp, w_dram, h_src, n_f_tiles, t_width, n_d_tiles=8):
    """Yields (ft, th, psum): out^T[f-tile] = sum_d W^T-tile . h_src tile."""
    wpool, psum = pp["wpool"], pp["psum_mm"]
    w_ap = w_dram.ap().rearrange("(dt dp) f -> dp dt f", dp=P)
    for c in range((n_f_tiles + 3) // 4):
        fw = min(512, (n_f_tiles - c * 4) * P)
        wc = wpool.tile([P, n_d_tiles, 512], F16, tag="w")
        nc.sync.dma_start(out=wc[:, :, :fw],
                          in_=w_ap[:, :, c * 512:c * 512 + fw])
        for fs in range(fw // P):
            ft = c * 4 + fs
            for th in range(t_width // 512):
                ps = psum.tile([P, 512], F32, tag="ps_mm")
                for dt in range(n_d_tiles):
                    nc.tensor.matmul(ps, wc[:, dt, fs * P:fs * P + P],
                                     h_src[:, dt, th * 512:th * 512 + 512],
                                     start=(dt == 0), stop=(dt == n_d_tiles - 1))
                yield ft, th, ps


def _vproj(nc, pp, w_dram, h_src, vt):
    """v[t, f] token-major with ones col at index 64: vt [128, 8, 16, 65].
    Yields after each (c, tt) slice (16 total) for interleaved emission."""
    wpool, psum = pp["wpool"], pp["psum_mm"]
    w_ap = w_dram.ap().rearrange("(dt dp) f -> dp dt f", dp=P)
    for c in range(2):
        wc = wpool.tile([P, 8, 512], F16, tag="w")
        nc.sync.dma_start(out=wc, in_=w_ap[:, :, c * 512:c * 512 + 512])
        for tt in range(8):
            ps = psum.tile([P, 512], F32, tag="ps_mm")
            for dt in range(8):
                nc.tensor.matmul(ps, h_src[:, dt, tt * P:tt * P + P],
                                 wc[:, dt, :], start=(dt == 0), stop=(dt == 7))
            nc.vector.tensor_copy(vt[:, tt, c * 8:c * 8 + 8, 0:HD],
                                  ps.rearrange("p (h d) -> p h d", h=8))
            yield


# fp16 Schraudolph exp on DVE: bits = round(1024*log2(e)*x + sigma),
# bitcast int16 -> fp16. Max rel err ~3%; softmax normalization cancels
# most of it (validated end-to-end: adds <1e-4 to final rel err).
# Input must be clamped to [-10, 9] first so the bits stay inside the
# finite-fp16 range [1, 31743] with no reliance on int16 saturation
# (exp(-10) ~ 4.5e-5 is 0 at softmax scale; scores never reach +9).
SEXP_A = 1477.3197
SEXP_B = 15316.5
SEXP_LO = -10.0
SEXP_HI = 9.0
AV_LAG = 3


def _attn_pair(nc, pp, hp, kT, vt, qT, out_sa, mk, tb, psum_sc, psum_av,
               dve_kts=()):
    """One head pair (2hp, 2hp+1) of attention, exp pipelined on ACT.
    Scores for the two heads are emitted adjacently (lhsT base partitions
    0/64) so the K=64 matmuls row-tile concurrently. av matmuls lag AV_LAG
    kts behind the scores so the PE never waits on ACT/DVE.
    mk: [128, 4, Q] fp16 0/1 causal mask for kt<4 (own half), or None.
    tb: [P,1] tail bias AP for kt>=4 (0 or -30000), or None.
    dve_kts: kts whose exp runs on DVE (Schraudolph); only for unmasked
    unbiased tiles (cross-attention)."""
    h0, h1 = 2 * hp, 2 * hp + 1
    tmp, sc_pool, bc = pp["tmp"], pp["stats"], pp["bcast"]
    avs = {}
    for h in (h0, h1):
        avs[h] = psum_av.tile([P, Q], F32, tag="av", name=f"av_{h}")
    pex = {}

    def emit_av(h, kt):
        nc.tensor.matmul(avs[h][0:HD + 1, :], vt[:, kt, h, :], pex.pop((h, kt)),
                         start=(kt == 0), stop=(kt == 7))

    for kt in range(8):
        scs = {}
        for h, fo in ((h0, 0), (h1, HD)):
            sc = psum_sc.tile([P, Q], F32, tag="sc", name=f"sc_{h}_{kt}")
            nc.tensor.matmul(sc, kT[fo:fo + HD, hp, kt * P:kt * P + P],
                             qT[fo:fo + HD, hp, :], start=True, stop=True)
            scs[h] = sc
        for h in (h0, h1):
            if kt in dve_kts:
                pxc = tmp.tile([P, Q], F32, tag="pexp_c", bufs=1,
                               name=f"pexpc_{h}_{kt}")
                nc.vector.tensor_scalar(pxc, scs[h], SEXP_HI, SEXP_LO,
                                        ALU.min, ALU.max)
                pxi = tmp.tile([P, Q], mybir.dt.int16, tag="pexp_i", bufs=4,
                               name=f"pexpi_{h}_{kt}")
                nc.vector.tensor_scalar(pxi, pxc, SEXP_A, SEXP_B,
                                        ALU.mult, ALU.add)
                pex[(h, kt)] = pxi.bitcast(F16)
                continue
            px_ = tmp.tile([P, Q], F16, tag="pexp", bufs=2 * AV_LAG + 2,
                           name=f"pexp_{h}_{kt}")
            if mk is not None and kt < 4:
                nc.scalar.activation(px_, scs[h], AFT.Exp)
                nc.vector.tensor_mul(px_, px_, mk[:, kt, :])
            elif tb is not None and kt >= 4:
                nc.scalar.activation(px_, scs[h], AFT.Exp, bias=tb)
            else:
                nc.scalar.activation(px_, scs[h], AFT.Exp)
            pex[(h, kt)] = px_
        if kt >= AV_LAG:
            for h in (h0, h1):
                emit_av(h, kt - AV_LAG)
    for kt in range(8 - AV_LAG, 8):
        for h in (h0, h1):
            emit_av(h, kt)

    for i, (h, fo) in enumerate(((h0, 0), (h1, HD))):
        # evict av to SBUF right away so the PSUM bank frees for the next
        # pair without waiting on the normalize chain; the denominator row
        # goes to its own partition-0 tile (custom-DVE recip and gpsimd
        # broadcast read from partition 0)
        avsb = tmp.tile([HD, Q], F32, tag="avsb", bufs=2, name=f"avsb_{h}")
        nc.vector.tensor_copy(avsb, avs[h][0:HD, :])
        dn = sc_pool.tile([1, Q], F32, tag="sc_c", name=f"dn_{h}")
        nc.vector.tensor_copy(dn, avs[h][HD:HD + 1, :])
        r = sc_pool.tile([1, Q], F32, tag="recip", name=f"r_{h}")
        nc.vector.reciprocal_approx_fast(r, dn)
        rb = bc.tile([HD, Q], F32, tag="rb", name=f"rb_{h}")
        nc.gpsimd.partition_broadcast(rb, r)
        nc.vector.tensor_mul(out_sa[fo:fo + HD, hp, :], avsb, rb)


def build_program():
    nc = bacc.Bacc("TRN2", target_bir_lowering=False, debug=False,
                   enable_asserts=False)

    din = lambda n, shape, dt_=F16: nc.declare_dram_parameter(
        n, shape, dt_, isOutput=False)
    x16 = din("x16", [D, L])             # fp16, rotated (stats + LN apply)
    xres = din("xres", [D, Q], F32)      # fp32 own half (residual stream)
    ctx16 = din("ctx16", [D, MCTX])
    maskT = din("maskT", [Q, Q])         # own-half causal 0/1 fp16, [keys, q]
    tbias = din("tbias", [P, 1], F32)    # 0 (s=1) or -30000 (s=0) tail bias
    WqT, WkT, WvT = din("WqT", [D, D]), din("WkT", [D, D]), din("WvT", [D, D])
    WsoT, Wq2T = din("WsoT", [D, D]), din("Wq2T", [D, D])
    Wk2T, Wv2T = din("Wk2T", [D, D]), din("Wv2T", [D, D])
    WcoT = din("WcoT", [D, D])
    W1T, W2T = din("W1T", [D, HID]), din("W2T", [HID, D])
    outT = nc.declare_dram_parameter("outT", [D, Q], F32, isOutput=True)

    es = {}
    with tile.TileContext(nc) as tc, ExitStack() as top:
        def popen(name, side=None, bufs=1, space=None):
            s = ExitStack()
            es[name] = s
            kwargs = dict(name=name, bufs=bufs)
            if side is not None:
                kwargs["side"] = side
            if space is not None:
                kwargs["space"] = space
            return s.enter_context(tc.tile_pool(**kwargs))

        def pclose(name):
            es.pop(name).close()

        const = top.enter_context(tc.tile_pool(name="const", bufs=1))
        wpool = top.enter_context(tc.tile_pool(name="wpool", bufs=2))
        tmp = top.enter_context(tc.tile_pool(name="tmp", bufs=2))
        stats = top.enter_context(tc.tile_pool(name="stats", bufs=1))
        bcast = top.enter_context(tc.tile_pool(name="bcast", bufs=2))
        psum_mm = top.enter_context(
            tc.tile_pool(name="psum_mm", bufs=2, space="PSUM"))

        ones = const.tile([P, 1], F16)
        nc.vector.memset(ones.bitcast(mybir.dt.uint16), 15360)
        eps_t = const.tile([1, 1], F32)
        nc.vector.memset(eps_t, EPS)
        tb_t = const.tile([P, 1], F32)
        nc.sync.dma_start(out=tb_t, in_=tbias[:, :])
        wjunk = const.tile([P, 512], F16)
        nc.vector.memset(wjunk.bitcast(mybir.dt.uint16), 0)

        pp = {"ones": ones, "eps": eps_t, "wpool": wpool, "tmp": tmp,
              "stats": stats, "bcast": bcast, "psum_mm": psum_mm}

        x16_r = x16.ap().rearrange("(dt dp) t -> dp dt t", dp=P)
        c16_r = ctx16.ap().rearrange("(dt dp) t -> dp dt t", dp=P)
        mask_r = maskT.ap().rearrange("(kt kp) q -> kp kt q", kp=P)

        # ---- warmup: keep PE busy + HAM warm through initial DMA/LN -------
        for wi in range(14):
            psw = psum_mm.tile([P, 512], F32, tag="ps_mm", name=f"warm{wi}")
            nc.tensor.matmul(psw[0:1, :], ones, wjunk, start=True, stop=True)

        # ---- phase A: norm1 + ctx norm + qkv ------------------------------
        pp["psum_stats"] = popen("psum_stats", space="PSUM")

        pcatt1 = popen("pcatt1", "left")
        k2T = pcatt1.tile([P, 8, MCTX], F16, tag="k2T")
        v2t = pcatt1.tile([P, 8, NH, HD + 1], F16, tag="v2t")
        nc.gpsimd.memset(v2t.bitcast(mybir.dt.uint16), 15360)
        phc = popen("phc", "left")
        hc = phc.tile([P, 8, MCTX], F16, tag="hc")
        pattn2 = popen("pattn2", "left")
        mk = pattn2.tile([P, 4, Q], F16, tag="mk")
        sa = pattn2.tile([P, 8, Q], F16, tag="sa")
        pattn1 = popen("pattn1", "left")
        qT = pattn1.tile([P, 8, Q], F16, tag="qT")
        kT = pattn1.tile([P, 8, L], F16, tag="kT")
        vt = pattn1.tile([P, 8, NH, HD + 1], F16, tag="vt")
        nc.gpsimd.memset(vt.bitcast(mybir.dt.uint16), 15360)
        pctx = popen("pctx", "left")
        cs16 = pctx.tile([P, 8, MCTX], F16, tag="cs16")
        px = popen("px", "left")
        xs = px.tile([P, 8, L], F16, tag="xs")
        nc.sync.dma_start(out=xs[:, :, 0:512], in_=x16_r[:, :, 0:512])
        nc.sync.dma_start(out=xs[:, :, 512:1024], in_=x16_r[:, :, 512:1024])
        nc.sync.dma_start(out=cs16, in_=c16_r)
        nc.sync.dma_start(out=mk, in_=mask_r)

        pht = popen("pht", "right")
        ht = pht.tile([P, 8, L], F16, tag="ht")
        _ln(nc, pp, xs, ht, L, "x")
        for wi in range(10):  # always-ready fillers: bridge the LN-chain
            psw = psum_mm.tile([P, 512], F32, tag="ps_mm", name=f"warmA{wi}")
            nc.tensor.matmul(psw[0:1, :], ones, wjunk, start=True, stop=True)
        pclose("px")
        # k first (self-attn pair 0 blocks on kT/qT ft0); evictions on ACT
        # (DVE carries the LN applies and vt evictions in this phase).
        # ctx LN emitted after the q evictions so B's first scores don't
        # queue behind its ACT squares/chain.
        for ft, th, ps in _proj(nc, pp, WkT, ht, 8, L):
            nc.scalar.copy(kT[:, ft, th * 512:th * 512 + 512], ps)
        for ft, th, ps in _proj(nc, pp, WqT, ht, 8, Q):
            nc.scalar.copy(qT[:, ft, :], ps)
        _ln(nc, pp, cs16, hc, MCTX, "c")
        pclose("pctx")
        for _ in _vproj(nc, pp, WvT, ht, vt):
            pass
        pclose("pht")
        pclose("psum_stats")

        # ---- phase B: self-attention, cross k/v interleaved ---------------
        k2_gen = _proj(nc, pp, Wk2T, hc, 8, MCTX)
        v2_gen = _vproj(nc, pp, Wv2T, hc, v2t)

        def k2_step(n):
            for _ in range(n):
                ft, th, ps = next(k2_gen, (None, None, None))
                if ps is None:
                    return
                nc.vector.tensor_copy(k2T[:, ft, th * 512:th * 512 + 512], ps)

        def v2_step(n):
            for _ in range(n):
                if next(v2_gen, "end") == "end":
                    return

        psum_sc = popen("psum_sc", space="PSUM", bufs=4)
        psum_av = popen("psum_av", space="PSUM", bufs=2)

        for hp in range(NH // 2):
            _attn_pair(nc, pp, hp, kT, vt, qT, sa, mk, tb_t, psum_sc, psum_av)
            # fill PE slack with cross k/v projection slices. NOTE: k2 and
            # v2 share the wpool "w" ring (bufs=2); consuming 2+2 per pair
            # keeps their chunk DMAs from reusing a buffer whose reads are
            # still being emitted (1+2 consumption raced and NaN'd).
            k2_step(2)
            v2_step(2)
        pclose("psum_av")
        pclose("psum_sc")
        pclose("pattn1")

        # ---- phase C: self out-proj + residual + LN(xa) + q2 --------------
        k2_step(16)
        v2_step(16)
        pp["psum_stats"] = psum_stats = popen("psum_stats", space="PSUM")
        ps_s = psum_stats.tile([1, Q], F32, tag="ps_s", name="ps_s_a")
        ps_q = psum_stats.tile([1, Q], F32, tag="ps_q", name="ps_q_a")

        pxa = popen("pxa", "right")
        resid = pxa.tile([P, 8, Q], F32, tag="resid")
        nc.sync.dma_start(
            out=resid, in_=xres.ap().rearrange("(dt dp) t -> dp dt t", dp=P))
        xa = pxa.tile([P, 8, Q], F32, tag="xa")
        xa16 = pxa.tile([P, 8, Q], F16, tag="xa16")
        stat_lag = []
        for ft, th, ps in _proj(nc, pp, WsoT, sa, 8, Q):
            nc.vector.tensor_add(xa[:, ft, :], ps, resid[:, ft, :])
            nc.scalar.copy(xa16[:, ft, :], xa[:, ft, :])
            sq = tmp.tile([P, Q], F16, tag="sq")
            nc.vector.tensor_mul(sq, xa16[:, ft, :], xa16[:, ft, :])
            stat_lag.append((ft, sq))
            if len(stat_lag) >= 2:  # lag stats MMs so PE doesn't stall
                lft, lsq = stat_lag.pop(0)
                nc.tensor.matmul(ps_s, ones, xa16[:, lft, :],
                                 start=(lft == 0), stop=(lft == 7))
                nc.tensor.matmul(ps_q, ones, lsq,
                                 start=(lft == 0), stop=(lft == 7))
        for lft, lsq in stat_lag:
            nc.tensor.matmul(ps_s, ones, xa16[:, lft, :],
                             start=(lft == 0), stop=(lft == 7))
            nc.tensor.matmul(ps_q, ones, lsq, start=(lft == 0), stop=(lft == 7))
        pclose("pattn2")
        pclose("phc")
        A_, B_ = _ln_chain(nc, pp, ps_s, ps_q, Q, "xa")
        pq2 = popen("pq2", "left")
        hq = pq2.tile([P, 8, Q], F16, tag="hq")
        for dt in range(8):
            t1 = tmp.tile([P, Q], F16, tag="lnap")
            nc.vector.tensor_mul(t1, xa16[:, dt, :], A_)
            nc.vector.tensor_add(hq[:, dt, :], t1, B_)
        q2T = pq2.tile([P, 8, Q], F16, tag="q2T")
        for ft, th, ps in _proj(nc, pp, Wq2T, hq, 8, Q):
            nc.scalar.copy(q2T[:, ft, :], ps)
        pclose("psum_stats")

        # ---- phase D: cross-attention, out-proj ft0/1 interleaved ---------
        psum_sc = popen("psum_sc", space="PSUM", bufs=4)
        psum_av = popen("psum_av", space="PSUM", bufs=2)
        ca = pq2.tile([P, 8, Q], F16, tag="ca")
        co_ap = WcoT.ap().rearrange("(dt dp) f -> dp dt f", dp=P)
        wc_co = wpool.tile([P, 8, 512], F16, tag="w", name="wc_co0")
        nc.sync.dma_start(out=wc_co, in_=co_ap[:, :, 0:512])
        co01 = [psum_mm.tile([P, Q], F32, tag="ps_mm", name=f"co{f}")
                for f in range(2)]

        def co_mm(dt):
            for f in range(2):
                nc.tensor.matmul(co01[f], wc_co[:, dt, f * P:f * P + P],
                                 ca[:, dt, :], start=(dt == 0), stop=(dt == 7))

        for hp in range(NH // 2):
            _attn_pair(nc, pp, hp, k2T, v2t, q2T, ca, None, None,
                       psum_sc, psum_av, dve_kts=(5,))
            if hp >= 1:  # lag: ca[:, hp-1, :] is settled by now
                co_mm(hp - 1)
        co_mm(7)
        pclose("psum_av")
        pclose("psum_sc")

        # ---- phase D2: cross out-proj rest + residual + LN(xb) ------------
        pp["psum_stats"] = psum_stats = popen("psum_stats", space="PSUM")
        ps_s = psum_stats.tile([1, Q], F32, tag="ps_s", name="ps_s_b")
        ps_q = psum_stats.tile([1, Q], F32, tag="ps_q", name="ps_q_b")
        pxb = popen("pxb", "right")
        xb = pxb.tile([P, 8, Q], F32, tag="xb")
        xb16 = pxb.tile([P, 8, Q], F16, tag="xb16")

        psum_co = popen("psum_co", space="PSUM", bufs=2)

        def co_rest():
            yield 0, co01[0]
            yield 1, co01[1]
            for ft in (2, 3):
                ps = psum_co.tile([P, Q], F32, tag="ps_co", name=f"co{ft}")
                for dt in range(8):
                    nc.tensor.matmul(ps, wc_co[:, dt, ft * P:ft * P + P],
                                     ca[:, dt, :],
                                     start=(dt == 0), stop=(dt == 7))
                yield ft, ps
            wc_co1 = wpool.tile([P, 8, 512], F16, tag="w", name="wc_co1")
            nc.sync.dma_start(out=wc_co1, in_=co_ap[:, :, 512:1024])
            for ft in (4, 5, 6, 7):
                ps = psum_co.tile([P, Q], F32, tag="ps_co", name=f"co{ft}")
                for dt in range(8):
                    nc.tensor.matmul(
                        ps, wc_co1[:, dt, (ft - 4) * P:(ft - 4) * P + P],
                        ca[:, dt, :], start=(dt == 0), stop=(dt == 7))
                yield ft, ps

        stat_lag = []
        for ft, ps in co_rest():
            nc.vector.tensor_add(xb[:, ft, :], ps, xa[:, ft, :])
            nc.scalar.copy(xb16[:, ft, :], xb[:, ft, :])
            sq = tmp.tile([P, Q], F16, tag="sq")
            nc.vector.tensor_mul(sq, xb16[:, ft, :], xb16[:, ft, :])
            stat_lag.append((ft, sq))
            if len(stat_lag) >= 2:
                lft, lsq = stat_lag.pop(0)
                nc.tensor.matmul(ps_s, ones, xb16[:, lft, :],
                                 start=(lft == 0), stop=(lft == 7))
                nc.tensor.matmul(ps_q, ones, lsq,
                                 start=(lft == 0), stop=(lft == 7))
        for lft, lsq in stat_lag:
            nc.tensor.matmul(ps_s, ones, xb16[:, lft, :],
                             start=(lft == 0), stop=(lft == 7))
            nc.tensor.matmul(ps_q, ones, lsq, start=(lft == 0), stop=(lft == 7))
        pclose("psum_co")
        pclose("pq2")
        pclose("pcatt1")

        A_, B_ = _ln_chain(nc, pp, ps_s, ps_q, Q, "xb")
        pmlp = popen("pmlp", "left")
        h2 = pmlp.tile([P, 8, Q], F16, tag="h2")
        for dt in range(8):
            t1 = tmp.tile([P, Q], F16, tag="lnap")
            nc.vector.tensor_mul(t1, xb16[:, dt, :], A_)
            nc.vector.tensor_add(h2[:, dt, :], t1, B_)
        pclose("psum_stats")

        # ---- phase E: MLP + streamed output -------------------------------
        gt = pmlp.tile([P, 32, Q], F16, tag="gt")
        for ft, th, ps in _proj(nc, pp, W1T, h2, 32, Q):
            nc.scalar.activation(gt[:, ft, :], ps, AFT.Gelu)

        out_r = outT.ap().rearrange("(dt dp) q -> dp dt q", dp=P)
        w2_ap = W2T.ap().rearrange("(dt dp) f -> dp dt f", dp=P)
        pw2 = popen("pw2", "left")
        for fh in range(4):
            pss = [psum_mm.tile([P, Q], F32, tag="ps_mm", name=f"fc2_{fh}_{e}")
                   for e in range(2)]
            for g in range(4):
                wc = pw2.tile([P, 8, 256], F16, tag="w2", bufs=3,
                              name=f"w2_{fh}_{g}")
                nc.sync.dma_start(
                    out=wc,
                    in_=w2_ap[:, g * 8:g * 8 + 8, fh * 256:fh * 256 + 256])
                for e in range(2):
                    for dt in range(8):
                        nc.tensor.matmul(pss[e], wc[:, dt, e * P:e * P + P],
                                         gt[:, g * 8 + dt, :],
                                         start=(g == 0 and dt == 0),
                                         stop=(g == 3 and dt == 7))
            for e in range(2):
                et = fh * 2 + e
                ot = tmp.tile([P, Q], F32, tag="ot", bufs=2, name=f"ot_{et}")
                nc.vector.tensor_add(ot, pss[e], xb[:, et, :])
                nc.sync.dma_start(out=out_r[:, et, :], in_=ot)
        pclose("pxb")
        pclose("pxa")
        pclose("pw2")
        pclose("pmlp")

    nc.compile()
    return nc


# ----------------------------------------------------------------------------
# host side
# ----------------------------------------------------------------------------

def _prep_inputs(x, context, sa_mask, W_qkv, W_self_out, W_q, W_kv, W_cross_out,
                 W_fc1, W_fc2, g_norm1, g_query_norm, g_context_norm, g_norm2):
    f32, f16 = np.float32, np.float16
    g1 = np.asarray(g_norm1, f32)[:, None]
    gq = np.asarray(g_query_norm, f32)[:, None]
    gc = np.asarray(g_context_norm, f32)[:, None]
    g2 = np.asarray(g_norm2, f32)[:, None]
    W_qkv = np.asarray(W_qkv, f32)
    W_kv = np.asarray(W_kv, f32)
    cw = lambda a: np.ascontiguousarray(a.astype(f16))
    weights = {
        "WqT": cw(W_qkv[0:D].T * g1 * f32(SCALE)),
        "WkT": cw(W_qkv[D:2 * D].T * g1),
        "WvT": cw(W_qkv[2 * D:3 * D].T * g1),
        "WsoT": cw(np.asarray(W_self_out, f32).T),
        "Wq2T": cw(np.asarray(W_q, f32).T * gq * f32(SCALE)),
        "Wk2T": cw(W_kv[0:D].T * gc),
        "Wv2T": cw(W_kv[D:2 * D].T * gc),
        "WcoT": cw(np.asarray(W_cross_out, f32).T),
        "W1T": cw(np.asarray(W_fc1, f32).T * g2),
        "W2T": cw(np.asarray(W_fc2, f32).T),
    }
    in_maps = []
    for c in range(8):
        b, s = c // 2, c % 2
        own = np.arange(s * Q, s * Q + Q)
        idx = np.concatenate([own, np.arange((1 - s) * Q, (1 - s) * Q + Q)])
        xb = np.asarray(x[b], f32)
        mask01 = np.where(np.asarray(sa_mask[b])[np.ix_(own, own)] == 0,
                          f16(0.0), f16(1.0))
        m = dict(weights)
        xr = np.ascontiguousarray(xb[idx].T)
        m["x16"] = xr.astype(f16)
        m["xres"] = np.ascontiguousarray(xr[:, 0:Q])
        m["maskT"] = np.ascontiguousarray(mask01.T.astype(f16))
        m["tbias"] = np.full((P, 1), NEG if s == 0 else 0.0, f32)
        m["ctx16"] = np.ascontiguousarray(
            np.asarray(context[b], f32).T.astype(f16))
        in_maps.append(m)
    return in_maps


def _check_mask(sa_mask):
    """Fast program assumes causal block structure across the two halves:
    second-half keys all-masked for first-half queries, all-open for
    second-half queries."""
    mask = np.asarray(sa_mask)
    lo, hi = np.arange(0, Q), np.arange(Q, L)
    for b in range(B):
        if not np.all(mask[b][np.ix_(lo, hi)] == 0):
            return False
        if not np.all(mask[b][np.ix_(hi, lo)] != 0):
            return False
    return True


def _gather(results, x_dtype):
    out = np.empty((B, L, D), np.float32)
    for c in range(8):
        b, s = c // 2, c % 2
        out[b, s * Q:(s + 1) * Q, :] = results[c]["outT"].T
    return out.astype(x_dtype, copy=False)


def _run(trace=False, **inputs):
    assert _check_mask(inputs["sa_mask"]), \
        "sa_mask does not have the expected causal block structure"
    if "nc" not in _CACHE:
        _CACHE["nc"] = build_program()
    nc = _CACHE["nc"]
    in_maps = _prep_inputs(**inputs)
    res = run_bass_kernel_spmd(nc, in_maps, list(range(8)), trace=trace)
    out = _gather(res.results, np.asarray(inputs["x"]).dtype)
    return out, res


def kernel(**inputs) -> np.ndarray:
    out, _ = _run(trace=False, **inputs)
    return out


def kernel_traced(**inputs):
    """Returns (output, exec_time_ns). Used by test.py."""
    import sys, types
    try:
        import antenv
        import trn_agent_boot.trn_boot as tb
        import concourse.bass_utils as bu
        if "antenv.axon_hooks" not in sys.modules:
            hook = tb._ntff_profile_via_ctypes('/opt/axon/libaxon_pjrt.so')
            mod = types.ModuleType("antenv.axon_hooks")
            mod.get_axon_ntff_profile_hook = lambda: hook
            mod.set_axon_ntff_profile_hook = lambda h: None
            sys.modules['antenv.axon_hooks'] = mod
            antenv.axon_hooks = mod
        bu.upload_artifacts = lambda tmpdir: "local://skipped"
    except Exception as e:
        print(f"ntff hook install failed: {e}")
    out, res = _run(trace=True, **inputs)
    return out, res.exec_time_ns
